# revision 51
# baseline (speedup 1.0000x reference)
"""GCN (3-layer, PyG GCNConv semantics) on 8 Trainium2 NeuronCores.

v5 strategy (v4 was 157us, v3 222us, baseline 1051us):
  Backward dependency slicing + decoupled gather DMA + single AllReduce.
  - Output is h3[mask] (G=100 rows).  L3 needs h2 only for S2 =
    in-neighbors(mask)+mask (~875 nodes); L2 needs h1 only for sources of
    S2's in-edges.  L2 is dst-sharded (one 128-row window per core); each
    core REDUNDANTLY computes exactly the ~1000 h1 rows its own L2 window
    consumes, so t1 stays core-local (no t1 collective).
  - dma_gather with prepare_only=True + trigger_dma: descriptor generation
    (the serial ~3.7ns/row GpSimd cost) no longer blocks on the DMA flight,
    and L2's desc-gen runs during L1 compute (the table RAW dep defers to
    the trigger).  Tail calls are split small across the 4 SWDGE queues so
    their flights overlap.
  - L3 is folded into L2's epilogue: each core multiplies its own A3 slice
    (host-precomputed sparse norm weights) against its t2 SBUF tile, then a
    single 128x256 fp32 AllReduce combines the partials; every core runs
    the tiny dense W3 head redundantly.  A dummy AllGather at t~0 absorbs
    the one-time collective-init barrier under L1 compute.
  - Self-loops are ordinary tokens weighted dis^2; each token carries the
    full norm dis[src]*dis[dst] in one scale vector; epilogues have no dis
    term.  L1 gathers read per-core COMPACT x tables (int16-safe).
"""

import numpy as np
import ml_dtypes

NEG = 0.01
CT = 16          # max gather tiles per dma_gather call
NQ = 4           # SWDGE queues


# ---------------------------------------------------------------- planner --
class Cfg:
    def __init__(self, N, E, G, IN, H, OUT, NCORES=8):
        self.N, self.E, self.G, self.IN, self.H, self.OUT = N, E, G, IN, H, OUT
        self.NC = NCORES


def _wrap16(idx):
    # idx [T] int -> [128, T/16] int16 (i at [i%16, i//16], replicated x8)
    a = idx.reshape(-1, 16).T
    return np.tile(a, (8, 1)).astype(np.int16).copy()


def _sched(NC, NW, percore):
    """Common window/token schedule.  percore[k] = (w, col, src, wt) arrays.

    Sorts each core's tokens window-major, pads every window to a common
    (max-over-cores) tile count, and scatters (src, wt, col) into padded
    token slots.  Pad slots: src=0, wt=0, col=128 (inert).
    """
    cnt = np.zeros((NC, NW), np.int64)
    for k, (w, col, srcv, wt) in enumerate(percore):
        cnt[k] += np.bincount(w, minlength=NW)
    TW = (cnt.max(axis=0) + 127) // 128
    TW = np.maximum(TW, 1)    # every window written so its rows are defined
    base = np.concatenate([[0], np.cumsum(TW * 128)])
    ntok = int(base[-1])
    nmm = int(TW.sum())
    w_of_mm = np.repeat(np.arange(NW), TW)

    cores = []
    for k, (w, col, srcv, wt) in enumerate(percore):
        order = np.lexsort((col, w))
        ws, cs = w[order], col[order]
        srcs, wts = srcv[order], wt[order]
        gidx = np.zeros(ntok, np.int64)
        scl = np.zeros(ntok, np.float32)
        colt = np.full(ntok, 128, np.int64)
        wcnt = np.bincount(ws, minlength=NW)
        offs = np.concatenate([[0], np.cumsum(wcnt)])[:-1]
        slot = base[:-1][ws] + (np.arange(len(ws)) - offs[ws])
        gidx[slot] = srcs
        scl[slot] = wts
        colt[slot] = cs
        cores.append({"gidx": gidx, "scl": scl, "col": colt})
    return dict(TW=TW, base=base, ntok=ntok, nmm=nmm, w_of_mm=w_of_mm,
                NW=NW, cores=cores)


def _balance(wgt, NW):
    """Greedy-balance weighted items into NW windows of <=128 slots each.
    Returns each item's local position (window*128 + slot).  Equalizing
    per-window token sums minimizes the max-over-cores tile padding."""
    import heapq
    order = np.argsort(-wgt, kind="stable")
    heap = [(0, w) for w in range(NW)]
    heapq.heapify(heap)
    counts = [0] * NW
    pos = np.empty(len(wgt), np.int64)
    for i in order:
        s, w = heapq.heappop(heap)
        while counts[w] >= 128:
            s, w = heapq.heappop(heap)
        pos[i] = w * 128 + counts[w]
        counts[w] += 1
        heapq.heappush(heap, (s + int(wgt[i]), w))
    return pos


def _call_sizes(nt, tail_split):
    """A small first call primes the gather->scale->matmul pipeline early,
    then big CT-tile calls, then the tail split across `tail_split` calls
    so their flights run on different queues concurrently."""
    sizes = []
    if nt > 2 * CT:
        sizes.append(CT // 2)
        nt -= CT // 2
    while nt > 3 * CT // 2:
        sizes.append(CT)
        nt -= CT
    while nt > 0:
        s = -(-nt // tail_split) if tail_split > 1 else nt
        s = max(1, min(s, nt))
        sizes.append(s)
        nt -= s
        tail_split = max(1, tail_split - 1)
    return sizes


def build_plan(cfg, edge_index, batch):
    NC, N = cfg.NC, cfg.N
    src = np.asarray(edge_index[0], np.int64)
    dst = np.asarray(edge_index[1], np.int64)
    deg = (np.bincount(dst, minlength=N) + 1.0)
    dis = (1.0 / np.sqrt(deg)).astype(np.float32)

    batch = np.asarray(batch, np.int64)
    mask = np.concatenate([[True], batch[1:] != batch[:-1]])
    M = np.nonzero(mask)[0]
    G = len(M)
    assert G <= 128

    flagM = np.zeros(N, bool); flagM[M] = True
    selM = flagM[dst]
    S2 = np.unique(np.concatenate([src[selM], M]))
    flag2 = np.zeros(N, bool); flag2[S2] = True
    sel2 = flag2[dst]

    n2 = len(S2)
    pos2 = np.full(N, -1, np.int64); pos2[S2] = np.arange(n2)
    posM = np.full(N, -1, np.int64); posM[M] = np.arange(G)
    L2C = -(-n2 // NC)
    LP2 = ((L2C + 127) // 128) * 128
    NW2 = LP2 // 128

    # edges sorted by dst, for fast per-node in-edge extraction
    eorder = np.argsort(dst, kind="stable")
    ds, ss = dst[eorder], src[eorder]

    def in_edges(nodes):
        lo = np.searchsorted(ds, nodes, "left")
        hi = np.searchsorted(ds, nodes, "right")
        cnt = hi - lo
        tot = int(cnt.sum())
        idx = np.repeat(lo, cnt) + (np.arange(tot) -
                                    np.repeat(np.cumsum(cnt) - cnt, cnt))
        return ss[idx], np.repeat(np.arange(len(nodes)), cnt)

    # ---- level 2 tokens, per core (src kept as GLOBAL node id for now)
    t2src = np.concatenate([src[sel2], S2])
    t2dstg = np.concatenate([dst[sel2], S2])
    t2dstp = pos2[t2dstg]
    t2wt = (dis[t2src] * dis[t2dstg]).astype(np.float32)
    core2 = t2dstp // L2C
    ln2 = t2dstp - core2 * L2C

    deg_in = np.bincount(dst, minlength=N)
    masks, Nks = [], []
    for k in range(NC):
        m = core2 == k
        masks.append(m)
        Nks.append(np.unique(t2src[m]))     # h1 rows this core must produce
    n1max = max(len(Nk) for Nk in Nks)
    LP1 = ((n1max + 127) // 128) * 128 if n1max else 128
    NW1 = LP1 // 128
    assert LP1 <= 32767

    lev1_pc, lev2_pc = [], []
    for k in range(NC):
        m, Nk = masks[k], Nks[k]
        posk = _balance(deg_in[Nk] + 1, NW1)   # balanced t1-table layout
        lev2_pc.append((ln2[m] // 128, ln2[m] % 128,
                        posk[np.searchsorted(Nk, t2src[m])], t2wt[m]))
        es, edp = in_edges(Nk)
        a_i = np.concatenate([edp, np.arange(len(Nk))])
        a_src = np.concatenate([es, Nk])
        a_p = posk[a_i]
        a_wt = (dis[a_src] * dis[Nk[a_i]]).astype(np.float32)
        lev1_pc.append((a_p // 128, a_p % 128, a_src, a_wt))

    lev1 = _sched(NC, NW1, lev1_pc)
    lev2 = _sched(NC, NW2, lev2_pc)
    lev1["LP"], lev2["LP"] = LP1, LP2

    # compact per-core x tables (int16-safe indices)
    XROWS = 0
    for c in lev1["cores"]:
        uniq = np.unique(c["gidx"])          # includes pad row id 0 (fine)
        c["xrows"] = uniq
        c["gidx"] = np.searchsorted(uniq, c["gidx"])
        XROWS = max(XROWS, len(uniq))
    XROWS = ((XROWS + 127) // 128) * 128
    assert XROWS <= 32767

    # ---- level 3: per-core sparse weight slices over local t2 windows
    def grow2(p):
        return (p // L2C) * LP2 + (p - (p // L2C) * L2C)
    A3 = np.zeros((NC * LP2, 128), np.float32)
    np.add.at(A3, (grow2(pos2[src[selM]]), posM[dst[selM]]),
              dis[src[selM]] * dis[dst[selM]])
    np.add.at(A3, (grow2(pos2[M]), np.arange(G)), dis[M] ** 2)

    # ---- gather-call schedule: (tok0, ntiles, level); queues round-robin
    calls = []
    for lv, lev, tsplit in ((0, lev1, NQ), (1, lev2, NQ)):
        j = 0
        for n in _call_sizes(lev["ntok"] // 128, tsplit):
            calls.append((j * 128, n, lv))
            j += n

    plan = {"lev1": lev1, "lev2": lev2, "XROWS": XROWS,
            "A3": A3, "G": G, "calls": calls,
            "ntok": lev1["ntok"] + lev2["ntok"],
            "nmm": lev1["nmm"] + lev2["nmm"]}
    per_core = [{} for _ in range(NC)]
    return plan, per_core


# ---------------------------------------------------------------- builder --
def build_bass(cfg, plan):
    import concourse.bacc as bacc
    import concourse.bass as bass
    import concourse.mybir as mybir
    from concourse.tile import TileContext
    from concourse.masks import make_identity
    from concourse import dve_ops
    from concourse.dve_spec import Spec, Src0, Src1, maxx, C2, lower
    from concourse.dve_uop import DveOpSpec
    from concourse.dve_spec import _has_src1 as has_src1

    def _mkop(name, spec):
        for op in dve_ops.OPS:
            if op.name == name:
                return op
        opcode = dve_ops._CUSTOM_DVE_ROW_BASE + len(dve_ops.OPS)
        dve_ops._SUB_OPCODE_FOR_NAME[name] = opcode
        uops_sha = {}
        for ver in ("v3", "v4"):
            try:
                sp = DveOpSpec(name=name, opcode=opcode,
                               uops=lower(spec, ver=ver),
                               rd1_en=has_src1(spec))
                uops_sha[ver] = sp.sha(ver)
            except Exception:
                pass
        op = dve_ops.DveOp(name, spec, subdim=False, uops_sha=uops_sha)
        dve_ops.OPS.append(op)
        dve_ops.CUSTOM_DVE_SPECS[name] = spec
        return op

    OPLEAKY = _mkop("GCN_LEAKY", Spec(
        body=maxx(Src0 + Src1, (Src0 + Src1) * C2),
        reference=lambda in0, in1, s0, s1, imm2: (
            np.maximum(in0 + in1, (in0 + in1) * imm2)),
    ))

    f32, bf16, i16 = mybir.dt.float32, mybir.dt.bfloat16, mybir.dt.int16
    IN, H, OUT = cfg.IN, cfg.H, cfg.OUT
    lev1, lev2 = plan["lev1"], plan["lev2"]
    XROWS, G = plan["XROWS"], plan["G"]
    LP1, LP2 = lev1["LP"], lev2["LP"]
    NW2 = lev2["NW"]
    NTOKT, NMMT = plan["ntok"], plan["nmm"]
    AC = mybir.ActivationFunctionType

    nc = bacc.Bacc("TRN2", target_bir_lowering=False, debug=False,
                   num_devices=cfg.NC, num_swdge_queues=NQ)

    xtabin = nc.dram_tensor("xtab", [XROWS, IN], bf16, kind="ExternalInput")
    w1in = nc.dram_tensor("w1", [IN, H], bf16, kind="ExternalInput")
    w2in = nc.dram_tensor("w2", [H, H], bf16, kind="ExternalInput")
    w3in = nc.dram_tensor("w3", [H, OUT], bf16, kind="ExternalInput")
    b1in = nc.dram_tensor("b1r", [128, H], f32, kind="ExternalInput")
    b2in = nc.dram_tensor("b2r", [128, H], f32, kind="ExternalInput")
    b3in = nc.dram_tensor("b3r", [128, OUT], f32, kind="ExternalInput")
    iotain = nc.dram_tensor("iota", [128, 128], bf16, kind="ExternalInput")
    colin = nc.dram_tensor("colmm", [128, NMMT], bf16, kind="ExternalInput")
    sclin = nc.dram_tensor("scl", [128, NTOKT // 128], f32,
                           kind="ExternalInput")
    gidxin = nc.dram_tensor("gidx", [128, NTOKT // 16], i16,
                            kind="ExternalInput")
    a3in = nc.dram_tensor("a3", [128, NW2 * 128], bf16, kind="ExternalInput")
    outt = nc.dram_tensor("out", [128, OUT], f32, kind="ExternalOutput")

    t1tab = nc.dram_tensor("t1tab", [LP1, H], bf16)
    ps3d = nc.dram_tensor("ps3d", [128, H], bf16)
    T3 = nc.dram_tensor("T3", [128, H], bf16, addr_space="Shared")

    rg = [list(range(cfg.NC))]
    callctr = [0]

    with TileContext(nc) as tc:
        with (
            tc.tile_pool(name="const", bufs=1) as constp,
            tc.tile_pool(name="msg", bufs=6) as msgp,
            tc.tile_pool(name="oh", bufs=4) as ohp,
            tc.tile_pool(name="small", bufs=4) as smallp,
            tc.tile_pool(name="packp", bufs=2) as packp,
            tc.tile_pool(name="psA", bufs=3, space="PSUM") as psA,
            tc.tile_pool(name="psT", bufs=2, space="PSUM") as psT,
            tc.tile_pool(name="psZ", bufs=2, space="PSUM") as psZ,
            tc.tile_pool(name="psB", bufs=1, space="PSUM") as psB,
        ):
            ident = constp.tile([128, 128], bf16)
            make_identity(nc, ident[:, :])
            iota_t = constp.tile([128, 128], bf16)
            nc.sync.dma_start(out=iota_t[:, :], in_=iotain[:, :])
            col_t = constp.tile([128, NMMT], bf16)
            nc.sync.dma_start(out=col_t[:, :], in_=colin[:, :])
            scl_t = constp.tile([128, NTOKT // 128], f32)
            nc.sync.dma_start(out=scl_t[:, :], in_=sclin[:, :])
            gidx_t = constp.tile([128, NTOKT // 16], i16)
            nc.sync.dma_start(out=gidx_t[:, :], in_=gidxin[:, :])
            a3_t = constp.tile([128, NW2 * 128], bf16)
            nc.sync.dma_start(out=a3_t[:, :], in_=a3in[:, :])
            w3_t = constp.tile([128, 2 * OUT], bf16)
            nc.sync.dma_start(
                out=w3_t[:, :].rearrange("p (ks f) -> p ks f", ks=2),
                in_=w3in.ap().rearrange("(ks p) f -> p ks f", p=128))
            b3_t = constp.tile([128, OUT], f32)
            nc.sync.dma_start(out=b3_t[:, :], in_=b3in[:, :])
            w1_t = constp.tile([IN, H], bf16)
            nc.sync.dma_start(out=w1_t[:, :], in_=w1in[:, :])
            w2_t = constp.tile([128, 2 * H], bf16)
            nc.sync.dma_start(
                out=w2_t[:, :].rearrange("p (ks f) -> p ks f", ks=2),
                in_=w2in.ap().rearrange("(ks p) f -> p ks f", p=128))
            b1_t = constp.tile([128, H], f32)
            nc.sync.dma_start(out=b1_t[:, :], in_=b1in[:, :])
            b2_t = constp.tile([128, H], f32)
            nc.sync.dma_start(out=b2_t[:, :], in_=b2in[:, :])

            def layer(lidx, lev, F, tab, tok_off, mm_off, u_epilogue):
                """one sweep: prepared gathers -> scale -> one-hot matmuls."""
                base, w_of_mm = lev["base"], lev["w_of_mm"]
                first_t = {w: int(base[w]) // 128 for w in range(lev["NW"])}
                last_t = {w: int(base[w + 1]) // 128 - 1
                          for w in range(lev["NW"])}
                psum_of = {}
                for (tok0, ntiles, lv) in plan["calls"]:
                    if lv != lidx:
                        continue
                    q = callctr[0] % NQ
                    msg = msgp.tile([128, CT * H], bf16, tag="msg",
                                    name=f"msg_{lidx}_{tok0}")
                    m3 = msg[:, 0:ntiles * F].rearrange(
                        "p (t f) -> p t f", f=F)
                    gt0 = tok_off + tok0
                    nc.gpsimd.dma_gather(
                        m3, tab,
                        gidx_t[:, gt0 // 16:(gt0 + ntiles * 128) // 16],
                        ntiles * 128, ntiles * 128, F,
                        single_packet=False, queue_num=q)
                    callctr[0] += 1
                    nc.vector.tensor_tensor(
                        out=m3, in0=m3,
                        in1=scl_t[:, gt0 // 128:gt0 // 128 + ntiles]
                            .rearrange("p (t a) -> p t a", a=1)
                            .broadcast_to([128, ntiles, F]),
                        op=mybir.AluOpType.mult)
                    oh_t = ohp.tile([128, CT * 128], bf16, tag="oh",
                                    name=f"oh_{lidx}_{tok0}")
                    lo = mm_off + tok0 // 128
                    nc.vector.tensor_tensor(
                        out=oh_t[:, 0:ntiles * 128].rearrange(
                            "p (k j) -> p k j", k=ntiles),
                        in0=iota_t[:, :].rearrange("p (a j) -> p a j", a=1)
                            .broadcast_to([128, ntiles, 128]),
                        in1=col_t[:, lo:lo + ntiles]
                            .rearrange("p (k a) -> p k a", a=1)
                            .broadcast_to([128, ntiles, 128]),
                        op=mybir.AluOpType.is_equal)
                    for j in range(ntiles):
                        g = tok0 // 128 + j          # global tile idx in level
                        w = int(w_of_mm[g])
                        st = (g == first_t[w])
                        sp = (g == last_t[w])
                        if st:
                            psum_of[w] = psA.tile([128, H], f32, tag="aggps",
                                                  name=f"ps_{lidx}_{w}")
                        nc.tensor.matmul(
                            psum_of[w][:, 0:F],
                            oh_t[:, bass.ts(j, 128)],
                            msg[:, j * F:(j + 1) * F],
                            start=st, stop=sp)
                        if sp:
                            u_epilogue(w, psum_of.pop(w))

            # ---------------- layer 1 (redundant per-consumer) -----------
            def epi1(w, ps):
                u = smallp.tile([128, IN], bf16, tag="u1", name=f"u1_{w}")
                nc.scalar.activation(u[:, :], ps[:, 0:IN], AC.Copy)
                pt = psT.tile([128, 128], bf16, tag="pt", name=f"pt1_{w}")
                nc.tensor.transpose(pt[:, :], u[:, :], ident[:, :])
                uT = smallp.tile([128, IN], bf16, tag="uT1", name=f"uT1_{w}")
                nc.scalar.activation(uT[:, :], pt[:, :], AC.Copy)
                pz = psZ.tile([128, H], f32, tag="pz", name=f"pz1_{w}")
                nc.tensor.matmul(pz[:, :], uT[:, :], w1_t[:, :],
                                 start=True, stop=True)
                t1 = smallp.tile([128, H], bf16, tag="t1o", name=f"t1o_{w}")
                nc.vector._custom_dve(
                    OPLEAKY, out=t1[:, :], in0=pz[:, :], in1=b1_t[:, :],
                    s0=0.0, s1=0.0, imm2=NEG)
                nc.sync.dma_start(
                    out=t1tab.ap().rearrange(
                        "(w p) f -> w p f", p=128)[w, :, :],
                    in_=t1[:, :])

            layer(0, lev1, IN, xtabin.ap(), 0, 0, epi1)

            # ---------------- layer 2 ----------------
            assert NW2 == 1

            psP = psB.tile([128, H], f32, tag="p3acc", name="psP")

            def epi2(w, ps):
                u = smallp.tile([128, H], bf16, tag="u2", name=f"u2_{w}")
                nc.scalar.activation(u[:, :], ps[:, :], AC.Copy)
                uT = smallp.tile([128, H], bf16, tag="uT2", name=f"uT2_{w}")
                for ks in range(2):
                    pt = psT.tile([128, 128], bf16, tag="pt",
                                  name=f"pt2_{w}_{ks}")
                    nc.tensor.transpose(pt[:, :], u[:, bass.ts(ks, 128)],
                                        ident[:, :])
                    nc.scalar.activation(uT[:, bass.ts(ks, 128)],
                                         pt[:, :], AC.Copy)
                pz = psZ.tile([128, H], f32, tag="pz", name=f"pz2_{w}")
                for ks in range(2):
                    nc.tensor.matmul(pz[:, :], uT[:, bass.ts(ks, 128)],
                                     w2_t[:, bass.ts(ks, H)],
                                     start=(ks == 0), stop=(ks == 1))
                t2 = smallp.tile([128, H], bf16, tag="t2o", name=f"t2o_{w}")
                nc.vector._custom_dve(
                    OPLEAKY, out=t2[:, :], in0=pz[:, :], in1=b2_t[:, :],
                    s0=0.0, s1=0.0, imm2=NEG)
                # folded L3 partial, TRANSPOSED so the post-AllReduce dense
                # head needs no PE transposes: psP[f_h, dst] += t2_h^T @ A3_w
                for hh in range(2):
                    nc.tensor.matmul(psP[:, bass.ts(hh, 128)],
                                     t2[:, bass.ts(hh, 128)],
                                     a3_t[:, bass.ts(w, 128)],
                                     start=(w == 0), stop=(w == NW2 - 1))
                if w == NW2 - 1:
                    p3 = packp.tile([128, H], bf16, tag="p3")
                    nc.scalar.activation(p3[:, :], psP[:, :], AC.Copy)
                    nc.sync.dma_start(out=ps3d.ap(), in_=p3[:, :])

            layer(1, lev2, H, t1tab.ap(), lev1["ntok"], lev1["nmm"], epi2)
            nc.gpsimd.collective_compute(
                "AllReduce", mybir.AluOpType.add, replica_groups=rg,
                ins=[ps3d.ap().opt()], outs=[T3.ap().opt()])

            # ---------------- dense W3 head (replicated) ----------------
            # T3 is already feature-major ([f, dst] halves) -> direct lhsT
            u3T = packp.tile([128, H], bf16, tag="u3T")
            nc.sync.dma_start(out=u3T[:, :], in_=T3.ap())
            ps4 = psZ.tile([128, OUT], f32, tag="pz", name="ps4")
            for ks in range(2):
                nc.tensor.matmul(ps4[:, :], u3T[:, bass.ts(ks, 128)],
                                 w3_t[:, bass.ts(ks, OUT)],
                                 start=(ks == 0), stop=(ks == 1))
            ot = packp.tile([128, OUT], f32, tag="ot")
            nc.vector.tensor_tensor(out=ot[:, :], in0=ps4[:, :],
                                    in1=b3_t[:, :],
                                    op=mybir.AluOpType.add)
            nc.sync.dma_start(out=outt[:, :], in_=ot[:, :])

    nc.finalize()
    return nc


# ----------------------------------------------------------------- driver --
def _make_inputs(cfg, plan, per_core, x, W1, b1, W2, b2, W3, b3):
    bf = ml_dtypes.bfloat16
    lev1, lev2 = plan["lev1"], plan["lev2"]
    XROWS = plan["XROWS"]
    LP2, NW2 = lev2["LP"], lev2["NW"]

    iota = np.tile(np.arange(128, dtype=np.float32)[None, :],
                   (128, 1)).astype(bf)
    b3r = np.tile(b3[None, :], (128, 1)).astype(np.float32)
    in_maps = []
    for k in range(cfg.NC):
        c1, c2 = lev1["cores"][k], lev2["cores"][k]
        xt = np.zeros((XROWS, cfg.IN), bf)
        xt[:len(c1["xrows"])] = x[c1["xrows"]].astype(bf)
        gidx = np.concatenate([c1["gidx"], c2["gidx"]])
        scl = np.concatenate([c1["scl"], c2["scl"]])
        col = np.concatenate([c1["col"], c2["col"]])
        # core k's own A3 slice, window-tiled, rows on partitions
        a3k = np.ascontiguousarray(
            plan["A3"][k * LP2:(k + 1) * LP2]
            .reshape(NW2, 128, 128).transpose(1, 0, 2).reshape(128, -1)
        ).astype(bf)
        in_maps.append({
            "xtab": xt,
            "w1": W1.astype(bf), "w2": W2.astype(bf), "w3": W3.astype(bf),
            "b1r": np.tile(b1[None, :], (128, 1)).astype(np.float32),
            "b2r": np.tile(b2[None, :], (128, 1)).astype(np.float32),
            "b3r": b3r,
            "iota": iota,
            "colmm": np.ascontiguousarray(
                col.reshape(-1, 128).T).astype(bf),
            "scl": np.ascontiguousarray(
                scl.reshape(-1, 128).T).astype(np.float32),
            "gidx": _wrap16(gidx),
            "a3": a3k,
        })
    return in_maps


def _assemble(cfg, plan, results):
    return results[0]["out"][:plan["G"], :cfg.OUT].astype(np.float32)


def kernel(x, edge_index, batch, W1, b1, W2, b2, W3, b3):
    from concourse.bass_utils import run_bass_kernel_spmd
    x = np.asarray(x)
    cfg = Cfg(N=x.shape[0], E=np.asarray(edge_index).shape[1],
              G=int(np.asarray(batch).max()) + 1,
              IN=x.shape[1], H=np.asarray(W2).shape[0],
              OUT=np.asarray(W3).shape[1])
    plan, per_core = build_plan(cfg, np.asarray(edge_index), np.asarray(batch))
    nc = build_bass(cfg, plan)
    in_maps = _make_inputs(cfg, plan, per_core, x,
                           np.asarray(W1), np.asarray(b1),
                           np.asarray(W2), np.asarray(b2),
                           np.asarray(W3), np.asarray(b3))
    res = run_bass_kernel_spmd(nc, in_maps, list(range(cfg.NC)))
    return _assemble(cfg, plan, res.results)
